# revision 19
# baseline (speedup 1.0000x reference)
"""Trainium2 Bass kernel for nn_PartialRadialLayer.

Math (see reference):
  ang    = arccos(cos(x, ray)) / pi                       [B]
  dec_n  = sigmoid(alpha_n * ang + beta_n)                [B, 255]
  dist   = soft-bin products down the depth-8 tree        [B, 256]
  out    = einsum('bl,bi,liw->bw', dist, x, T)            [B, 32]

Device strategy (pure data parallel over 8 cores, 8192 rows each):
  * angle via 0.5 - arctan(dot / sqrt(ss*rn2 - dot^2))/pi (no arccos LUT)
  * decisions per batch tile as a rank-2 PE matmul
    z = [ang; 1].T @ [alpha; beta] followed by an ACT sigmoid
  * tree->leaf products via a level cascade in batch-major layout
    using P*(1-g) = P - P*g (two DVE ops per level, 16 tiles at a time)
  * main contraction re-associated as U[b,(w,i)] = dist[b,:] @ T2 on the
    PE (K=256, fp16), then out[b,w] = sum_i x[b,i]*U[b,(w,i)] via an ACT
    PSUM->SBUF fp16 copy, a DVE multiply against a DMA-broadcast x tile
    (16-bit 2x mode) and a strided fp16 reduce (2x).
  * xbar transposes (dist -> dist.T tiles) ride the ACT HWDGE queue,
    bulk copies ride the SP queue.
"""

import numpy as np

B = 65536
NCORES = 8
BC = B // NCORES          # 8192 rows per core
I = 64
W = 32
L = 256
NT = BC // 128            # 64 batch tiles of 128 rows
GRP = 16                  # tiles per cascade group
EPS = 1e-8

# ----------------------------------------------------------------------------
# Environment workarounds (old walrus build in this image)
# ----------------------------------------------------------------------------

def _install_fixups():
    import orjson
    import concourse.tile as tile
    import concourse.mybir as mybir
    import concourse.bass2jax as bass2jax
    import concourse.bass_utils as bass_utils
    from concourse.vector_clock import ScopedClock

    if getattr(tile.TileContext, "_ant_fixups_installed", False):
        return

    # 1. Tail drain: at most one sync-wait per CTRL instruction.
    def _drain_and_barrier(self, tick_clock, wait_clock):
        drain_inst = self.nc.sync.drain()
        wait_clock.add_sem_waits(
            drain_inst.ins, ScopedClock({None: tick_clock.global_clock})
        )
        si = drain_inst.ins.sync_info
        waits = list(si.on_wait) if si is not None else []
        if len(waits) > 1:
            drain_inst.ins.sync_info = mybir.SyncInfo(
                on_wait=waits[:1], on_update=list(si.on_update)
            )
            for k in range(1, len(waits)):
                extra = self.nc.sync.drain()
                extra.ins.sync_info = mybir.SyncInfo(
                    on_wait=waits[k : k + 1], on_update=[]
                )
        self.nc.all_engine_barrier()
        popped = self.nc._tile_sem_poison_stack.pop()
        assert popped is self._sem_poison
        self.nc.clear_and_free_semaphores(list(self.sems.allocated().values()))
        self.nc.all_engine_barrier()

    tile.TileContext._drain_and_barrier = _drain_and_barrier
    tile.TileContext._ant_fixups_installed = True

    # 2. Split multi-wait instructions onto same-engine NoOps in the BIR.
    def _split_multiwait_bir(bir_bytes):
        d = orjson.loads(bir_bytes)
        for fn in d.get("functions", []):
            for blk in fn.get("blocks", []):
                out = []
                for inst in blk["instructions"]:
                    si = inst.get("sync_info")
                    waits = (si or {}).get("on_wait") or []
                    if len(waits) > 1 and inst.get("engine") not in (
                        None,
                        "Unassigned",
                    ):
                        for k, w in enumerate(waits[:-1]):
                            nop = {
                                "name": f"{inst['name']}-sw{k}",
                                "engine": inst["engine"],
                                "opcode": "NoOp",
                                "ins": [],
                                "outs": [],
                                "sync_info": {"on_wait": [w], "on_update": []},
                            }
                            if inst.get("debug") is not None:
                                nop["debug"] = inst["debug"]
                            out.append(nop)
                        si["on_wait"] = [waits[-1]]
                    out.append(inst)
                blk["instructions"] = out
        return orjson.dumps(d)

    orig = bass_utils.compile_bir_kernel

    def patched(bir_json, tmpdir, neff_name="file.neff"):
        return orig(_split_multiwait_bir(bytes(bir_json)), tmpdir, neff_name)

    bass_utils.compile_bir_kernel = patched
    bass2jax.compile_bir_kernel = patched


# ----------------------------------------------------------------------------
# Device program
# ----------------------------------------------------------------------------

_prog_cache = {}


def _build_program():
    if "nc" in _prog_cache:
        return _prog_cache["nc"]
    _install_fixups()
    import concourse.bass as bass
    import concourse.tile as tile
    import concourse.mybir as mybir

    f32, f16 = mybir.dt.float32, mybir.dt.float16
    AF = mybir.ActivationFunctionType
    ALU = mybir.AluOpType

    nc = bass.Bass("TRN2", target_bir_lowering=False, debug=False,
                   num_devices=NCORES)

    xs_d = nc.dram_tensor("xs", [BC, I], f32, kind="ExternalInput").ap()
    x16_d = nc.dram_tensor("x16", [BC, I], f16, kind="ExternalInput").ap()
    t2_d = nc.dram_tensor("t2", [2, 128, W * I], f16, kind="ExternalInput").ap()
    rayrep_d = nc.dram_tensor("rayrep", [128, 16 * I], f32,
                              kind="ExternalInput").ap()
    ab_d = nc.dram_tensor("ab", [2, 256], f32, kind="ExternalInput").ap()
    ones_d = nc.dram_tensor("ones8k", [1, BC], f32, kind="ExternalInput").ap()
    pp_d = nc.dram_tensor("pp", [128, 8], f32, kind="ExternalInput").ap()
    out_d = nc.dram_tensor("out", [BC, W], f32, kind="ExternalOutput").ap()
    ang_d = nc.dram_tensor("angd", [128, NT], f32).ap()  # internal scratch

    with tile.TileContext(nc) as tc:
        with (
            tc.tile_pool(name="const", bufs=1) as constp,
            tc.tile_pool(name="persist", bufs=1) as persist,
            tc.tile_pool(name="loop", bufs=2) as loopp,
            tc.tile_pool(name="loopsm", bufs=4) as loopsm,
            tc.tile_pool(name="casc", bufs=2) as cascp,
        ):
            # ---- constants ----
            t2_0 = constp.tile([128, W * I], f16, tag="t2_0")
            t2_1 = constp.tile([128, W * I], f16, tag="t2_1")
            nc.sync.dma_start(t2_0[:], t2_d[0])
            nc.sync.dma_start(t2_1[:], t2_d[1])
            pp = constp.tile([128, 8], f32, tag="pp")
            nc.sync.dma_start(pp[:], pp_d[:])
            ab = constp.tile([2, 256], f32, tag="ab")
            nc.sync.dma_start(ab[:], ab_d[:])
            x16 = constp.tile([128, NT * I], f16, tag="x16")
            nc.sync.dma_start(
                x16[:].rearrange("j (c i) -> j c i", i=I),
                x16_d.rearrange("(c j) i -> j c i", j=128),
            )

            # ---- stage A: angles (chunks of 16 t-columns) ----
            with tc.tile_pool(name="stagea", bufs=2) as sa, \
                 tc.tile_pool(name="stats", bufs=1) as sstat:
                rayrep = sstat.tile([128, 16 * I], f32, tag="rayrep")
                nc.sync.dma_start(rayrep[:], rayrep_d[:])
                st = sstat.tile([128, NT, 8], f32, tag="stats")
                xs3 = xs_d.rearrange("(p t) i -> p t i", p=128)
                for ch in range(NT // 16):
                    tsl = slice(ch * 16, (ch + 1) * 16)
                    XSc = sa.tile([128, 16 * I], f32, tag="XSc")
                    nc.sync.dma_start(
                        XSc[:].rearrange("p (t i) -> p t i", i=I),
                        xs3[:, tsl, :],
                    )
                    tmpc = sa.tile([128, 16 * I], f32, tag="tmpc")
                    nc.vector.tensor_mul(tmpc[:], XSc[:], XSc[:])
                    nc.vector.reduce_sum(
                        st[:, tsl, 0],
                        tmpc[:].rearrange("p (t i) -> p t i", i=I),
                        axis=mybir.AxisListType.X,
                    )
                    nc.vector.tensor_mul(tmpc[:], XSc[:], rayrep[:])
                    nc.vector.reduce_sum(
                        st[:, tsl, 1],
                        tmpc[:].rearrange("p (t i) -> p t i", i=I),
                        axis=mybir.AxisListType.X,
                    )
                ss = st[:, :, 0]
                dot = st[:, :, 1]
                d2 = st[:, :, 2]
                q = st[:, :, 3]
                s = st[:, :, 4]
                rinv = st[:, :, 5]
                v = st[:, :, 6]
                at = st[:, :, 7]
                nc.vector.tensor_mul(d2, dot, dot)
                # q = max(ss*rn2 - dot^2, tiny)
                nc.vector.scalar_tensor_tensor(
                    q, ss, pp[:, 4:5], d2, op0=ALU.mult, op1=ALU.subtract
                )
                nc.vector.tensor_scalar_max(q, q, 1e-20)
                nc.scalar.activation(s, q, AF.Sqrt)
                nc.vector.reciprocal(rinv, s)
                nc.vector.tensor_mul(v, dot, rinv)
                nc.scalar.activation(at, v, AF.Arctan)
                ANG = sstat.tile([128, NT], f32, tag="ANG")
                # ang = 0.5 - arctan(v)/pi
                nc.scalar.activation(
                    ANG[:], at, AF.Copy, bias=0.5, scale=float(-1.0 / np.pi)
                )
                nc.sync.dma_start(ang_d[:, :], ANG[:])

            # ---- decisions: rank-2 matmul + sigmoid per tile ----
            DEC = persist.tile([128, NT * 256], f16, tag="DEC")
            angl = constp.tile([2, BC], f32, tag="angl")
            nc.sync.dma_start(angl[0:1, :], ang_d.flatten().unsqueeze(0))
            nc.sync.dma_start(angl[1:2, :], ones_d[:])
            with tc.tile_pool(name="zps", bufs=4, space="PSUM") as zps:
                for c2 in range(NT // 2):
                    z2 = zps.tile([128, 512], f32, tag="z")
                    for h in range(2):
                        c = 2 * c2 + h
                        nc.tensor.matmul(
                            z2[:, h * 256 : (h + 1) * 256],
                            angl[:, c * 128 : (c + 1) * 128], ab[:],
                            start=True, stop=True,
                        )
                    nc.scalar.activation(
                        DEC[:, c2 * 512 : (c2 + 1) * 512], z2[:], AF.Sigmoid
                    )

            # ---- per group: cascade then main tiles ----
            DIST = persist.tile([128, NT * 256], f16, tag="DIST")
            ones16 = constp.tile([128, GRP], f16, tag="P0")
            nc.gpsimd.memset(ones16[:], 1.0)
            x16_3 = x16[:].rearrange("j (c i) -> j c i", i=I)

            with tc.tile_pool(name="ups", bufs=2, space="PSUM") as ups:
                for g in range(NT // GRP):
                    c0 = g * GRP
                    # tree cascade for this group of tiles
                    Pprev = ones16
                    for d in range(1, 9):
                        n_half = 1 << (d - 1)
                        n_full = 1 << d
                        node0 = n_half - 1
                        if d == 8:
                            Pd = DIST[:, c0 * 256 : (c0 + GRP) * 256]
                        else:
                            pd_t = cascp.tile([128, GRP * n_full], f16,
                                              tag=f"P{d}")
                            Pd = pd_t[:]
                        out3 = Pd.rearrange(
                            "p (c k two) -> p (c k) two", two=2, k=n_half
                        )
                        evens = out3[:, :, 0].rearrange(
                            "p (c k) -> p c k", k=n_half
                        )
                        odds = out3[:, :, 1].rearrange(
                            "p (c k) -> p c k", k=n_half
                        )
                        prev3 = Pprev[:].rearrange(
                            "p (c k) -> p c k", k=n_half
                        )
                        dec3 = DEC[:, c0 * 256 : (c0 + GRP) * 256].rearrange(
                            "p (c n) -> p c n", n=256
                        )[:, :, node0 : node0 + n_half]
                        nc.vector.tensor_mul(evens, prev3, dec3)
                        nc.vector.tensor_sub(odds, prev3, evens)
                        Pprev = Pd

                    # main per-tile work
                    for c in range(c0, c0 + GRP):
                        dT_lo = loopsm.tile([128, 128], f16, tag="dTlo")
                        dT_hi = loopsm.tile([128, 128], f16, tag="dThi")
                        # xbar transposes on the ACT HWDGE queue
                        tq = nc.sync if c % 2 == 0 else nc.scalar
                        tq.dma_start(
                            dT_lo[:], DIST[:, c * 256 : c * 256 + 128],
                            transpose=True,
                        )
                        tq.dma_start(
                            dT_hi[:], DIST[:, c * 256 + 128 : (c + 1) * 256],
                            transpose=True,
                        )
                        U = ups.tile([128, W * I], f32, tag="U")
                        for nq in range(4):
                            sl = slice(nq * 512, (nq + 1) * 512)
                            nc.tensor.matmul(
                                U[:, sl], dT_lo[:], t2_0[:, sl],
                                start=True, stop=False,
                            )
                            nc.tensor.matmul(
                                U[:, sl], dT_hi[:], t2_1[:, sl],
                                start=False, stop=True,
                            )
                        Ub = loopp.tile([128, W * I], f16, tag="Ub")
                        nc.scalar.activation(Ub[:], U[:], AF.Copy)
                        XB = loopp.tile([128, W * I], f16, tag="XB")
                        xq = nc.scalar if c % 2 == 0 else nc.sync
                        xq.dma_start(
                            XB[:].rearrange("p (w i) -> p w i", i=I),
                            x16_3[:, c, :].unsqueeze(1).broadcast_to(
                                (128, W, I)
                            ),
                        )
                        Mx = loopp.tile([128, W, I], f16, tag="Mx")
                        nc.vector.tensor_mul(
                            Mx[:].rearrange("p w i -> p (w i)"),
                            Ub[:], XB[:],
                        )
                        t32 = loopsm.tile([128, W, 32], f16, tag="t32")
                        nc.vector.tensor_add(
                            t32[:], Mx[:, :, 0:32], Mx[:, :, 32:64]
                        )
                        t16 = loopsm.tile([128, W, 16], f16, tag="t16")
                        nc.vector.tensor_add(
                            t16[:], t32[:, :, 0:16], t32[:, :, 16:32]
                        )
                        outc = loopsm.tile([128, W], f32, tag="outc")
                        nc.vector.reduce_sum(
                            outc[:], t16[:], axis=mybir.AxisListType.X,
                        )
                        nc.sync.dma_start(
                            out_d.rearrange("(c j) w -> c j w", j=128)[c],
                            outc[:],
                        )

    _prog_cache["nc"] = nc
    return nc


# ----------------------------------------------------------------------------
# Host wrapper
# ----------------------------------------------------------------------------

def _host_prep(x, ray, inner_transforms, w_i, b_i, a_i):
    x = np.asarray(x, dtype=np.float32)
    ray = np.asarray(ray, dtype=np.float32)
    T = np.asarray(inner_transforms, dtype=np.float32)
    w_i = np.asarray(w_i, dtype=np.float32)
    b_i = np.asarray(b_i, dtype=np.float32)
    a_i = np.asarray(a_i, dtype=np.float32)

    def sig(z):
        return 1.0 / (1.0 + np.exp(-z))

    alpha = ((0.5 + sig(w_i)) * (1.0 + a_i))[0]      # [255]
    beta = (-sig(b_i) * (1.0 + a_i))[0]              # [255]
    rn = max(float(np.linalg.norm(ray[0])), EPS)
    rn2 = rn * rn

    ab = np.zeros((2, 256), dtype=np.float32)
    ab[0, :255] = alpha
    ab[1, :255] = beta
    ab[1, 255] = -30.0  # dec -> 0, never used

    pp = np.zeros((128, 8), dtype=np.float32)
    pp[:, 4] = rn2

    # T2[l, w*64+i] = T[l, i, w]; split into two l-halves
    T2 = np.ascontiguousarray(
        T.transpose(0, 2, 1).reshape(L, W * I)
    ).astype(np.float16).reshape(2, 128, W * I)

    rayrep = np.tile(ray[0], (128, 16)).astype(np.float32)  # [128, 16*I]
    x16 = x.astype(np.float16)
    ones8k = np.ones((1, BC), dtype=np.float32)
    return x, x16, T2, rayrep, ab, pp, ones8k


def _in_maps(x, x16, T2, rayrep, ab, pp, ones8k):
    maps = []
    for cid in range(NCORES):
        sl = slice(cid * BC, (cid + 1) * BC)
        maps.append({
            "xs": np.ascontiguousarray(x[sl]),
            "x16": np.ascontiguousarray(x16[sl]),
            "t2": T2,
            "rayrep": rayrep,
            "ab": ab,
            "pp": pp,
            "ones8k": ones8k,
        })
    return maps


def kernel(x, ray, inner_transforms, w_i, b_i, a_i):
    from concourse.bass_utils import run_bass_kernel_spmd

    prep = _host_prep(x, ray, inner_transforms, w_i, b_i, a_i)
    nc = _build_program()
    res = run_bass_kernel_spmd(nc, _in_maps(*prep),
                               core_ids=list(range(NCORES)))
    out = np.concatenate([res.results[c]["out"] for c in range(NCORES)], axis=0)
    return out.astype(np.float32)


def run_traced(inputs):
    """For test.py: same as kernel() but with NTFF tracing; returns
    (output, BassKernelResults)."""
    from concourse.bass_utils import run_bass_kernel_spmd

    prep = _host_prep(**inputs)
    nc = _build_program()
    res = run_bass_kernel_spmd(
        nc, _in_maps(*prep), core_ids=list(range(NCORES)), trace=True
    )
    out = np.concatenate([res.results[c]["out"] for c in range(NCORES)], axis=0)
    return out.astype(np.float32), res
